# revision 1
# baseline (speedup 1.0000x reference)
"""Trainium2 kernel for nn_MultiHeadClassifier.

Math: out[i] = W[task_labels[i]] @ x[i] + b[task_labels[i]]
  x [262144, 1024] f32, task_labels [262144] int, W [8, 32, 1024], b [8, 32]

Strategy (8 NeuronCores, data-parallel over batch):
  - Each core gets 32768 rows. x is staged in HBM transposed
    ([8, 128, 32768]: k-tile, d-within-tile, row) so the PE can contract
    over d (partition dim) directly.
  - All T=8 heads are computed at once per 128-row tile: 8 float32r
    matmuls (full PE rate, ~1.5e-4 rel err) accumulate y = x @ Wflat.T
    ([128 rows, 256]) in PSUM, plus a K=1 bf16 matmul adding the bias.
  - Head selection (the MoE routing) happens on-device on the DVE:
    one-hot mask [128, 8] broadcast-multiplied into y viewed [128, 8, 32],
    then a strided reduce over the 8 task slots -> out tile [128, 32].
  - Output is written in [128, 256, 32] (partition-major) layout with
    fully contiguous per-partition DMA runs; host reshapes back.
"""

import sys

sys.path.insert(0, "/opt/trn_rl_repo")

import numpy as np
import ml_dtypes

import concourse.bass as bass
import concourse.tile as tile
from concourse import bacc, mybir
from concourse import bass_utils

B, D, C, T = 262144, 1024, 32, 8
NCORES = 8
N = B // NCORES  # 32768 rows per core
P = 128
KO = D // P  # 8 contraction tiles
TC = T * C  # 256 = all-heads output width
SB = 1024  # rows per superblock (one x DMA)
NT = N // P  # 256 row-tiles per core
SBT = SB // P  # row-tiles per superblock
NSB = N // SB  # superblocks per core

# set by test harness to collect a profile; harness-invoked kernel() keeps it off
TRACE = False
LAST_RESULTS = None


def _build():
    f32 = mybir.dt.float32
    f32r = mybir.dt.float32r
    bf16 = mybir.dt.bfloat16

    nc = bacc.Bacc("TRN2", debug=False, num_devices=NCORES)
    # xt[sb, ki, ko, r]: one superblock is a contiguous 2 MB region with
    # 16 KB contiguous per partition -> near-peak DMA efficiency.
    xt_d = nc.dram_tensor("xt", [NSB, P, KO, SB], f32r, kind="ExternalInput")
    wft_d = nc.dram_tensor("wft", [KO, P, TC], f32r, kind="ExternalInput")
    mask_d = nc.dram_tensor("mask8", [P, NT, T], f32, kind="ExternalInput")
    # bpack[0, :P] = ones, bpack[0, P:] = b.reshape(256) twice (bf16)
    bpack_d = nc.dram_tensor("bpack", [1, P + 2 * TC], bf16, kind="ExternalInput")
    out_d = nc.dram_tensor("out", [P, NT, C], f32, kind="ExternalOutput")

    with tile.TileContext(nc) as tc:
        with (
            tc.tile_pool(name="consts", bufs=1) as consts,
            tc.tile_pool(name="xpool", bufs=5) as xpool,
            tc.tile_pool(name="work", bufs=8) as work,
            tc.tile_pool(name="opool", bufs=3) as opool,
            tc.tile_pool(name="psum", bufs=8, space="PSUM") as psum,
        ):
            # first x superblock in flight before the consts
            xts0 = xpool.tile([P, KO, SB], f32r, tag="xts")
            nc.sync.dma_start(xts0[:], xt_d[0])

            # consts on the ACT ring: the SP ring stays a pure x stream
            wft = consts.tile([P, KO, TC], f32r)
            nc.scalar.dma_start(wft[:], wft_d[:].rearrange("ko ki n -> ki ko n"))
            mask8 = consts.tile([P, NT, T], f32)
            nc.scalar.dma_start(mask8[:], mask_d[:])
            bpack = consts.tile([1, P + 2 * TC], bf16)
            nc.scalar.dma_start(bpack[:], bpack_d[:])
            ones1 = bpack[:, :P]
            bexp2 = bpack[:, P:]  # [1, 512] = b flat, twice

            # Engine warmups: with the 1-sync-wait-per-instruction ISA
            # limit, give each engine one instruction that observes the
            # const DMA lanes, so steady-state instructions carry at most
            # one wait each.
            scratch = psum.tile([P, TC], mybir.dt.float32, tag="y")
            w0 = wft[:, 0, :1].bitcast(bf16)  # [P, 2] garbage bf16 view
            nc.tensor.matmul(scratch[:2, :2], w0, w0, start=True, stop=True)
            dve_scr = work.tile([P, T], f32, tag="dve_scr")
            nc.vector.tensor_copy(dve_scr[:], mask8[:, 0, :])

            for sb in range(NSB):
                if sb == 0:
                    xts = xts0
                else:
                    xts = xpool.tile([P, KO, SB], f32r, tag="xts")
                    nc.sync.dma_start(xts[:], xt_d[sb])
                out_sb = opool.tile([P, SBT, C], f32, tag="out_sb")
                for st in range(SBT):
                    ro = sb * SBT + st
                    y = psum.tile([P, TC], mybir.dt.float32, tag="y")
                    # bias first: absorbs the psum-slot WAR wait; single
                    # const producer (bpack DMA).
                    nc.tensor.matmul(
                        y[:], ones1, bexp2[:, :TC], start=True, stop=False
                    )
                    for ko in range(KO):
                        nc.tensor.matmul(
                            y[:],
                            xts[:, ko, st * P : (st + 1) * P],
                            wft[:, ko, :],
                            start=False,
                            stop=(ko == KO - 1),
                        )
                    # tmp[p, t, c] = y[p, t*C+c] * mask8[p, ro, t]
                    tmp = work.tile([P, TC], f32, tag="tmp")
                    nc.vector.tensor_tensor(
                        tmp[:].rearrange("p (t c) -> p t c", t=T),
                        y[:].rearrange("p (t c) -> p t c", t=T),
                        mask8[:, ro, :, None].to_broadcast((P, T, C)),
                        mybir.AluOpType.mult,
                    )
                    # out[p, c] = sum_t tmp[p, t, c]
                    nc.vector.tensor_reduce(
                        out_sb[:, st, :],
                        tmp[:].rearrange("p (t c) -> p c t", t=T),
                        axis=mybir.AxisListType.X,
                        op=mybir.AluOpType.add,
                    )
                # out on the ACT HWDGE ring so it never delays xts loads
                # queued on the SP ring
                nc.scalar.dma_start(
                    out_d[:, sb * SBT : (sb + 1) * SBT, :], out_sb[:]
                )
    nc.compile()
    return nc


_NC = None


def _get_nc():
    global _NC
    if _NC is None:
        _NC = _build()
    return _NC


def kernel(x, task_labels, W, b):
    global LAST_RESULTS
    x = np.asarray(x)
    if x.dtype != np.float32:
        x = x.astype(np.float32)
    labels = np.asarray(task_labels).astype(np.int32)
    W = np.asarray(W)
    if W.dtype != np.float32:
        W = W.astype(np.float32)
    b = np.asarray(b)
    if b.dtype != np.float32:
        b = b.astype(np.float32)

    wft = np.ascontiguousarray(W.reshape(TC, D).T).reshape(KO, P, TC)
    bpack = (
        np.concatenate(
            [np.ones(P, np.float32), b.reshape(TC), b.reshape(TC)]
        )
        .reshape(1, P + 2 * TC)
        .astype(ml_dtypes.bfloat16)
    )
    tids = np.arange(T, dtype=np.int32)[None, None, :]

    in_maps = []
    for c in range(NCORES):
        xs = x[c * N : (c + 1) * N]
        ls = labels[c * N : (c + 1) * N]
        # xt[sb, ki, ko, r] = xs[sb*SB + r, ko*P + ki]
        xt = np.ascontiguousarray(
            xs.reshape(NSB, SB, KO, P).transpose(0, 3, 2, 1)
        )
        lab2 = ls.reshape(NT, P).T  # [P, NT]
        mask8 = (lab2[:, :, None] == tids).astype(np.float32)
        in_maps.append(
            {"xt": xt, "wft": wft, "mask8": mask8, "bpack": bpack}
        )

    nc = _get_nc()
    res = bass_utils.run_bass_kernel_spmd(
        nc, in_maps, core_ids=list(range(NCORES)), trace=TRACE
    )
    LAST_RESULTS = res
    outs = [
        r["out"].transpose(1, 0, 2).reshape(N, C) for r in res.results
    ]
    return np.concatenate(outs, axis=0)



# revision 2
# speedup vs baseline: 1.9519x; 1.9519x over previous
"""Trainium2 kernel for nn_MultiHeadClassifier.

Math: out[i] = W[task_labels[i]] @ x[i] + b[task_labels[i]]
  x [262144, 1024] f32, task_labels [262144] int, W [8, 32, 1024], b [8, 32]

Strategy (8 NeuronCores, data-parallel over batch), v2:
  - The problem is HBM-bound: the only large tensor is x. v1 streamed x as
    f32 (128 MiB/core) and computed all 8 heads on the PE, selecting via a
    one-hot mask (8x the needed matmul work). v2 halves the traffic and
    cuts PE work 8x:
      * x is cast to bf16 on the host (tolerance is 2e-2; bf16 adds ~2e-3).
      * Rows are routed on the host: each core's 32768 rows are placed
        into 8 static 4096-row blocks by task id. The device schedule is
        fully static: rows in block t use W[t]. No masks, no padding.
      * Block overflow (a task with >4096 rows on one core; ~24 rows
        expected per block) is computed on the host in numpy and patched
        into the output. Underfull blocks hold zero rows (harmless).
  - Device inner loop: per 512-row chunk, 8 accumulating matmuls with the
    block's W as the stationary operand ([128k, 32], N=512 moving rows
    from the [ki, rows]-transposed x), psum [32, 512] -> DVE copy/cast to
    bf16 -> per-superblock DMA out as [32, rows].
  - Host: inverse permutation, bias add, f32 cast.
"""

import sys

sys.path.insert(0, "/opt/trn_rl_repo")

import numpy as np
import ml_dtypes

import concourse.bass as bass
import concourse.tile as tile
from concourse import bacc, mybir
from concourse import bass_utils

B, D, C, T = 262144, 1024, 32, 8
NCORES = 8
N = B // NCORES  # 32768 rows per core
P = 128
KO = D // P  # 8 contraction tiles
BLK = N // T  # 4096 rows per task block (static capacity)
SB = 2048  # rows per superblock (one x DMA = 4 MB)
NSB = N // SB  # 16 superblocks per core
CHUNK = 512  # rows per psum accumulation group
NCH = SB // CHUNK  # chunks per superblock

# set by test harness to collect a profile; harness-invoked kernel() keeps it off
TRACE = False
LAST_RESULTS = None


def _build():
    f32 = mybir.dt.float32
    bf16 = mybir.dt.bfloat16

    nc = bacc.Bacc("TRN2", debug=False, num_devices=NCORES)
    # xt[sb, ki, ko, r]: rows already routed into task blocks; 32 KB
    # contiguous per partition per superblock -> near-peak DMA efficiency.
    xt_d = nc.dram_tensor("xt", [NSB, P, KO, SB], bf16, kind="ExternalInput")
    # wall[ki, t, ko, c] = W[t, c, ko*128+ki] (lhsT layout, all 8 heads)
    wall_d = nc.dram_tensor("wall", [P, T, KO, C], bf16, kind="ExternalInput")
    out_d = nc.dram_tensor("out", [C, N], bf16, kind="ExternalOutput")

    with tile.TileContext(nc) as tc:
        with (
            tc.tile_pool(name="consts", bufs=1) as consts,
            tc.tile_pool(name="xpool", bufs=4) as xpool,
            tc.tile_pool(name="opool", bufs=3) as opool,
            tc.tile_pool(name="psum", bufs=8, space="PSUM") as psum,
        ):
            # first x superblock in flight before the consts
            xts0 = xpool.tile([P, KO, SB], bf16, tag="xts")
            nc.sync.dma_start(xts0[:], xt_d[0])

            # consts on the ACT ring: the SP ring stays a pure x stream
            wall = consts.tile([P, T, KO, C], bf16)
            nc.scalar.dma_start(wall[:], wall_d[:])

            # Engine warmups: one instruction per engine that observes the
            # const DMA lane, so steady-state instructions carry at most
            # one semaphore wait each.
            scratch = psum.tile([C, CHUNK], f32, tag="y")
            nc.tensor.matmul(
                scratch[:2, :2], wall[:2, 0, 0, :2], wall[:2, 0, 0, :2],
                start=True, stop=True,
            )
            dve_scr = opool.tile([P, C], bf16, tag="dve_scr")
            nc.vector.tensor_copy(dve_scr[:], wall[:, 0, 0, :])

            for sb in range(NSB):
                if sb == 0:
                    xts = xts0
                else:
                    xts = xpool.tile([P, KO, SB], bf16, tag="xts")
                    nc.sync.dma_start(xts[:], xt_d[sb])
                out_sb = opool.tile([C, SB], bf16, tag="out_sb")
                for st in range(NCH):
                    t = (sb * SB + st * CHUNK) // BLK  # static task id
                    y = psum.tile([C, CHUNK], f32, tag="y")
                    for ko in range(KO):
                        nc.tensor.matmul(
                            y[:],
                            wall[:, t, ko, :],
                            xts[:, ko, st * CHUNK : (st + 1) * CHUNK],
                            start=(ko == 0),
                            stop=(ko == KO - 1),
                        )
                    nc.vector.tensor_copy(
                        out_sb[:, st * CHUNK : (st + 1) * CHUNK], y[:]
                    )
                # out on the ACT HWDGE ring so it never delays xts loads
                nc.scalar.dma_start(
                    out_d[:, sb * SB : (sb + 1) * SB], out_sb[:]
                )
    nc.compile()
    return nc


_NC = None


def _get_nc():
    global _NC
    if _NC is None:
        _NC = _build()
    return _NC


def kernel(x, task_labels, W, b):
    global LAST_RESULTS
    x = np.asarray(x)
    if x.dtype != np.float32:
        x = x.astype(np.float32)
    labels = np.asarray(task_labels).astype(np.int64)
    W32 = np.asarray(W)
    if W32.dtype != np.float32:
        W32 = W32.astype(np.float32)
    b32 = np.asarray(b)
    if b32.dtype != np.float32:
        b32 = b32.astype(np.float32)

    wall = np.ascontiguousarray(
        W32.reshape(T, C, KO, P).transpose(3, 0, 2, 1)
    ).astype(ml_dtypes.bfloat16)

    in_maps = []
    placements = []
    for c in range(NCORES):
        lab = labels[c * N : (c + 1) * N]
        xs16 = x[c * N : (c + 1) * N].astype(ml_dtypes.bfloat16)
        slot_to_row = np.full(N, -1, np.int64)
        overflow = []
        for t in range(T):
            idx = np.nonzero(lab == t)[0]
            n_place = min(len(idx), BLK)
            slot_to_row[t * BLK : t * BLK + n_place] = idx[:n_place]
            if len(idx) > BLK:
                overflow.append(idx[BLK:])
        placed = slot_to_row >= 0
        xb = np.zeros((N, D), ml_dtypes.bfloat16)
        xb[placed] = xs16[slot_to_row[placed]]
        # xt[sb, ki, ko, r] = xb[sb*SB + r, ko*P + ki]
        xt = np.ascontiguousarray(
            xb.reshape(NSB, SB, KO, P).transpose(0, 3, 2, 1)
        )
        in_maps.append({"xt": xt, "wall": wall})
        placements.append(
            (
                slot_to_row,
                placed,
                np.concatenate(overflow) if overflow else np.empty(0, np.int64),
            )
        )

    nc = _get_nc()
    res = bass_utils.run_bass_kernel_spmd(
        nc, in_maps, core_ids=list(range(NCORES)), trace=TRACE
    )
    LAST_RESULTS = res

    out = np.empty((B, C), np.float32)
    for c in range(NCORES):
        dev = np.asarray(res.results[c]["out"]).astype(np.float32).T  # [N, C]
        slot_to_row, placed, overflow = placements[c]
        rows = slot_to_row[placed]
        out[c * N + rows] = dev[placed]
        if len(overflow):
            lab = labels[c * N : (c + 1) * N]
            xs = x[c * N : (c + 1) * N]
            for t in np.unique(lab[overflow]):
                rr = overflow[lab[overflow] == t]
                out[c * N + rr] = xs[rr] @ W32[t].T
    out += b32[labels]
    return out


# revision 5
# speedup vs baseline: 2.3626x; 1.2104x over previous
"""Trainium2 kernel for nn_MultiHeadClassifier.

Math: out[i] = W[task_labels[i]] @ x[i] + b[task_labels[i]]
  x [262144, 1024] f32, task_labels [262144] int, W [8, 32, 1024], b [8, 32]

Strategy (8 NeuronCores, data-parallel over batch), v3:
  - The problem is HBM-bound: the only large tensor is x. v1 streamed x as
    f32 (128 MiB/core) and computed all 8 heads on the PE, selecting via a
    one-hot mask (8x the needed matmul work). v2+ halves the traffic and
    cuts PE work 8x:
      * x is cast to bf16 on the host (tolerance is 2e-2; bf16 adds ~2.6e-3).
      * Rows are routed on the host: each core's 32768 rows are placed
        into 8 static 4096-row blocks by task id. The device schedule is
        fully static: rows in block t use W[t]. No masks, no padding.
      * Block overflow (a task with >4096 rows on one core; ~24 rows
        expected per block) is computed on the host in numpy and patched
        into the output. Underfull blocks hold zero rows (harmless).
  - Device inner loop: per 512-row chunk, 8 accumulating matmuls with the
    block's W as the stationary operand ([128k, 32], N=512 moving rows
    from the [ki, rows]-transposed x), psum [32, 512] -> DVE copy/cast to
    bf16 -> per-superblock DMA out as [32, rows].
  - v3: the last superblock is streamed as 4 chunk-sized DMAs with
    per-chunk output DMAs so the pipeline tail overlaps the final bytes'
    arrival; consts load first so the PE warms up early.
  - Host: inverse permutation, bias add, f32 cast.
"""

import sys

sys.path.insert(0, "/opt/trn_rl_repo")

import numpy as np
import ml_dtypes

import concourse.bass as bass
import concourse.tile as tile
from concourse import bacc, mybir
from concourse import bass_utils

B, D, C, T = 262144, 1024, 32, 8
NCORES = 8
N = B // NCORES  # 32768 rows per core
P = 128
KO = D // P  # 8 contraction tiles
BLK = N // T  # 4096 rows per task block (static capacity)
SB = 2048  # rows per superblock (one x DMA = 4 MB)
NSB = N // SB  # 16 superblocks per core
NSB1 = NSB - 1  # full superblocks; the last one streams per chunk
CHUNK = 512  # rows per psum accumulation group
NCH = SB // CHUNK  # chunks per superblock
NROW1 = NSB1 * SB  # rows covered by full superblocks

# set by test harness to collect a profile; harness-invoked kernel() keeps it off
TRACE = False
LAST_RESULTS = None


def _build():
    f32 = mybir.dt.float32
    bf16 = mybir.dt.bfloat16

    nc = bacc.Bacc("TRN2", debug=False, num_devices=NCORES)
    # xt[sb, ki, ko, r]: rows already routed into task blocks; 32 KB
    # contiguous per partition per superblock -> near-peak DMA efficiency.
    xt_d = nc.dram_tensor("xt", [NSB1, P, KO, SB], bf16, kind="ExternalInput")
    # last superblock, chunk-major so the tail streams at 512-row grain
    xtl_d = nc.dram_tensor(
        "xtl", [NCH, P, KO, CHUNK], bf16, kind="ExternalInput"
    )
    # wall[ki, t, ko, c] = W[t, c, ko*128+ki] (lhsT layout, all 8 heads)
    wall_d = nc.dram_tensor("wall", [P, T, KO, C], bf16, kind="ExternalInput")
    out_d = nc.dram_tensor("out", [C, N], bf16, kind="ExternalOutput")

    with tile.TileContext(nc) as tc:
        with (
            tc.tile_pool(name="sbuf", bufs=1) as sbuf,
            tc.tile_pool(name="xpool", bufs=4) as xpool,
            tc.tile_pool(name="psum", bufs=8, space="PSUM") as psum,
        ):
            # consts first on the ACT ring (small, so the PE warmup isn't
            # stuck behind x superblocks); the SP ring is a pure x stream
            wall = sbuf.tile([P, T, KO, C], bf16)
            nc.scalar.dma_start(wall[:], wall_d[:])

            xts0 = xpool.tile([P, KO, SB], bf16, tag="xts")
            nc.sync.dma_start(xts0[:], xt_d[0])

            # Engine warmups: one instruction per engine that observes the
            # const DMA lane, so steady-state instructions carry at most
            # one semaphore wait each.
            scratch = psum.tile([C, CHUNK], f32, tag="y")
            nc.tensor.matmul(
                scratch[:2, :2], wall[:2, 0, 0, :2], wall[:2, 0, 0, :2],
                start=True, stop=True,
            )
            dve_scr = sbuf.tile([P, C], bf16, tag="dve_scr")
            nc.vector.tensor_copy(dve_scr[:], wall[:, 0, 0, :])

            def chunk_group(y, xap, t, out_slice):
                for ko in range(KO):
                    nc.tensor.matmul(
                        y[:],
                        wall[:, t, ko, :],
                        xap(ko),
                        start=(ko == 0),
                        stop=(ko == KO - 1),
                    )
                nc.vector.tensor_copy(out_slice, y[:])

            for sb in range(NSB1):
                if sb == 0:
                    xts = xts0
                else:
                    xts = xpool.tile([P, KO, SB], bf16, tag="xts")
                    nc.sync.dma_start(xts[:], xt_d[sb])
                out_sb = xpool.tile([C, SB], bf16, tag="out_sb")
                for st in range(NCH):
                    t = (sb * SB + st * CHUNK) // BLK  # static task id
                    y = psum.tile([C, CHUNK], f32, tag="y")
                    chunk_group(
                        y,
                        lambda ko: xts[:, ko, st * CHUNK : (st + 1) * CHUNK],
                        t,
                        out_sb[:, st * CHUNK : (st + 1) * CHUNK],
                    )
                # out on the ACT HWDGE ring so it never delays xts loads
                nc.scalar.dma_start(
                    out_d[:, sb * SB : (sb + 1) * SB], out_sb[:]
                )

            # tail: per-chunk stream of the last superblock
            for st in range(NCH):
                xtl = xpool.tile([P, KO, CHUNK], bf16, tag="xtl")
                nc.sync.dma_start(xtl[:], xtl_d[st])
                t = (NROW1 + st * CHUNK) // BLK
                y = psum.tile([C, CHUNK], f32, tag="y")
                out_l = xpool.tile([C, CHUNK], bf16, tag="out_l")
                chunk_group(y, lambda ko: xtl[:, ko, :], t, out_l[:])
                nc.scalar.dma_start(
                    out_d[
                        :, NROW1 + st * CHUNK : NROW1 + (st + 1) * CHUNK
                    ],
                    out_l[:],
                )
    nc.compile()
    return nc


_NC = None


def _get_nc():
    global _NC
    if _NC is None:
        _NC = _build()
    return _NC


def kernel(x, task_labels, W, b):
    global LAST_RESULTS
    x = np.asarray(x)
    if x.dtype != np.float32:
        x = x.astype(np.float32)
    labels = np.asarray(task_labels).astype(np.int64)
    W32 = np.asarray(W)
    if W32.dtype != np.float32:
        W32 = W32.astype(np.float32)
    b32 = np.asarray(b)
    if b32.dtype != np.float32:
        b32 = b32.astype(np.float32)

    wall = np.ascontiguousarray(
        W32.reshape(T, C, KO, P).transpose(3, 0, 2, 1)
    ).astype(ml_dtypes.bfloat16)

    in_maps = []
    placements = []
    for c in range(NCORES):
        lab = labels[c * N : (c + 1) * N]
        xs16 = x[c * N : (c + 1) * N].astype(ml_dtypes.bfloat16)
        slot_to_row = np.full(N, -1, np.int64)
        overflow = []
        for t in range(T):
            idx = np.nonzero(lab == t)[0]
            n_place = min(len(idx), BLK)
            slot_to_row[t * BLK : t * BLK + n_place] = idx[:n_place]
            if len(idx) > BLK:
                overflow.append(idx[BLK:])
        placed = slot_to_row >= 0
        xb = np.zeros((N, D), ml_dtypes.bfloat16)
        xb[placed] = xs16[slot_to_row[placed]]
        # xt[sb, ki, ko, r] = xb[sb*SB + r, ko*P + ki]
        xt = np.ascontiguousarray(
            xb[:NROW1].reshape(NSB1, SB, KO, P).transpose(0, 3, 2, 1)
        )
        xtl = np.ascontiguousarray(
            xb[NROW1:].reshape(NCH, CHUNK, KO, P).transpose(0, 3, 2, 1)
        )
        in_maps.append({"xt": xt, "xtl": xtl, "wall": wall})
        placements.append(
            (
                slot_to_row,
                placed,
                np.concatenate(overflow) if overflow else np.empty(0, np.int64),
            )
        )

    nc = _get_nc()
    res = bass_utils.run_bass_kernel_spmd(
        nc, in_maps, core_ids=list(range(NCORES)), trace=TRACE
    )
    LAST_RESULTS = res

    out = np.empty((B, C), np.float32)
    for c in range(NCORES):
        dev = np.asarray(res.results[c]["out"]).astype(np.float32).T  # [N, C]
        slot_to_row, placed, overflow = placements[c]
        rows = slot_to_row[placed]
        out[c * N + rows] = dev[placed]
        if len(overflow):
            lab = labels[c * N : (c + 1) * N]
            xs = x[c * N : (c + 1) * N]
            for t in np.unique(lab[overflow]):
                rr = overflow[lab[overflow] == t]
                out[c * N + rr] = xs[rr] @ W32[t].T
    out += b32[labels]
    return out
